# revision 7
# baseline (speedup 1.0000x reference)
"""Fused masked-attention kernel for Trainium2, data-parallel over batch on 8 cores.

v16 design notes (all per core; one batch element per core):
- Mask ships mostly as fp8e4 {0,1} (halves mask DMA vs f16) and is applied by
  one of four routes, tile-by-tile (knobs N_PE/N_POOL/N_DVE8/N_F16):
    PE route:   ps = 320*I @ m_fp8 (start) + scores (accum); ACT does
                exp(0.125*x - 40)  -> masked entries become exp(s/8-40) ~ 0.
    Pool route: gpsimd tensor_copy converts m fp8->f16 ahead of time; DVE does
                the f16 2x-mode multiply.
    DVE-fp8:    DVE tensor_mul with the fp8 operand directly (1x mode).
    DVE-f16:    mask tile shipped as f16 over DMA; DVE 2x multiply.
- AV runs as row-split concurrent pairs (K=64 halves on disjoint PE row
  groups) into two PSUM accumulators p_oA/p_oB, combined at qb end via
  copy+add on DVE.
- Startup streams K/V/Q: only K0,K1 + V0 + q0 + mask0 load before the main
  loop; remaining K/V blocks DMA+project inside qb0's slots ahead of their
  first use, so ACT starts ~15us in.
- Steady state: ACT runs the pure exp stream (the wall at ~1.13us/tile);
  PE holds scores+AV+mask-adds+projections under that beat; DVE/Pool/DMA
  split the rest of the mask work.
"""

import numpy as np
import ml_dtypes

import concourse.bass as bass
import concourse.tile as tile
from concourse import bacc, mybir
from concourse import bass_utils

B, L, E, H = 8, 4096, 1024, 64
NCORES = 8
F32 = mybir.dt.float32
F16 = mybir.dt.float16
F8 = mybir.dt.float8e4

LB = 512           # q-block and projection block width
NQB = L // LB      # 8
NCH = L // 128     # 32 k-chunks
NEC = E // 128     # 8 e-chunks
NG = 16            # tiles (chunk pairs) per q-block

# mask route knobs (per qb, sum must be NG)
N_PE = 4           # mask added on PE via 320*I @ m_fp8
N_POOL = 5         # fp8 -> f16 on Pool, f16 mult on DVE
N_DVE8 = 2         # direct fp8 mult on DVE
N_F16 = 5          # f16 mask over DMA, f16 mult on DVE
assert N_PE + N_POOL + N_DVE8 + N_F16 == NG
NF8C = 2 * (N_PE + N_POOL + N_DVE8)   # fp8 chunks per qb
NF16C = 2 * N_F16                     # f16 chunks per qb

MB = 320.0         # mask bias weight: exp(0.125*(s+320m) - 40) = m?exp(s/8):~0


def build_nc():
    nc = bacc.Bacc(
        "TRN2",
        target_bir_lowering=False,
        debug=False,
        enable_asserts=False,
        num_devices=NCORES,
    )
    q2 = nc.dram_tensor("q2", [NQB, 128, NEC, LB], F16, kind="ExternalInput").ap()
    k2 = nc.dram_tensor("k2", [NQB, 128, NEC, LB], F16, kind="ExternalInput").ap()
    v2 = nc.dram_tensor("v2", [NQB, 128, NEC, LB], F16, kind="ExternalInput").ap()
    m8 = nc.dram_tensor("m8", [NQB, 128, NF8C, LB], F8, kind="ExternalInput").ap()
    m16 = nc.dram_tensor("m16", [NQB, 128, NF16C, LB], F16, kind="ExternalInput").ap()
    wqD = nc.dram_tensor("wqD", [E, 128], F16, kind="ExternalInput").ap()
    wkD = nc.dram_tensor("wkD", [E, 128], F16, kind="ExternalInput").ap()
    wvT = nc.dram_tensor("wvT", [E, H], F16, kind="ExternalInput").ap()
    ident = nc.dram_tensor("ident", [64, 64], F16, kind="ExternalInput").ap()
    id320 = nc.dram_tensor("id320", [128, 128], F16, kind="ExternalInput").ap()
    out = nc.dram_tensor("out", [H + 1, L], F32, kind="ExternalOutput").ap()

    EXP = mybir.ActivationFunctionType.Exp

    with tile.TileContext(nc) as tc:
        with (
            tc.tile_pool(name="const", bufs=1) as constp,
            tc.tile_pool(name="persist", bufs=1) as persist,
            tc.tile_pool(name="kin", bufs=2) as kinp,
            tc.tile_pool(name="vin", bufs=2) as vinp,
            tc.tile_pool(name="qin", bufs=2) as qinp,
            tc.tile_pool(name="m8p", bufs=2) as m8pool,
            tc.tile_pool(name="m16p", bufs=2) as m16pool,
            tc.tile_pool(name="mcv", bufs=5) as mcvpool,
            tc.tile_pool(name="pt", bufs=8) as ptpool,
            tc.tile_pool(name="osb", bufs=2) as opool,
            tc.tile_pool(name="ps_st", bufs=2, space="PSUM") as ps_st,
            tc.tile_pool(name="ps_oa", bufs=1, space="PSUM") as ps_oa,
            tc.tile_pool(name="ps_ob", bufs=1, space="PSUM") as ps_ob,
            tc.tile_pool(name="ps_pj", bufs=1, space="PSUM") as ps_pj,
            tc.tile_pool(name="ps_tr", bufs=1, space="PSUM") as ps_tr,
        ):
            # ---- constants / weights ----
            wq_sb = constp.tile([128, NEC, 128], F16)
            wk_sb = constp.tile([128, NEC, 128], F16)
            wv_sb = constp.tile([128, NEC, H], F16)
            nc.sync.dma_start(wq_sb[:], wqD.rearrange("(c p) h -> p c h", p=128))
            nc.sync.dma_start(wk_sb[:], wkD.rearrange("(c p) h -> p c h", p=128))
            nc.sync.dma_start(wv_sb[:], wvT.rearrange("(c p) h -> p c h", p=128))
            id_sb = constp.tile([64, 64], F16)
            nc.sync.dma_start(id_sb[:], ident)
            id320_sb = constp.tile([128, 128], F16)
            nc.sync.dma_start(id320_sb[:], id320)
            bneg40 = constp.tile([128, 1], F32)
            nc.vector.memset(bneg40[:], -40.0)

            # persistent projected tensors
            QT_sb = persist.tile([128, L], F16)   # rows 0:64 = Q^T, 64:128 copy
            KT_sb = persist.tile([128, L], F16)
            VT_sb = persist.tile([64, L], F16)    # V^T staging
            V_sb = persist.tile([128, NCH, 128], F16)  # [k, h] + ones col 64
            nc.vector.memset(V_sb[:, :, H : 128], 0.0)
            nc.vector.memset(V_sb[:, :, H : H + 1], 1.0)

            # masks for qb0
            m8_sb0 = m8pool.tile([128, NF8C, LB], F8, tag="m8")
            nc.sync.dma_start(m8_sb0[:], m8[0])
            m16_sb0 = m16pool.tile([128, NF16C, LB], F16, tag="m16")
            nc.sync.dma_start(m16_sb0[:], m16[0])

            # ---- PE warmup on weights (HAM) ----
            p_w = ps_st.tile([128, 1024], F32, tag="p_st")
            for w in range(110):
                nc.tensor.matmul(
                    p_w[:, 0:128], wq_sb[:, 0, :], wq_sb[:, 0, 0:128],
                    start=True, stop=True,
                )

            def proj_k_block(b, k_in):
                ls = b * LB
                p_pj = ps_pj.tile([128, LB], F32, tag="pj")
                for ec in range(NEC):
                    nc.tensor.matmul(
                        p_pj[:], wk_sb[:, ec, :], k_in[:, ec, :],
                        start=(ec == 0), stop=(ec == NEC - 1),
                    )
                nc.vector.tensor_copy(KT_sb[:, ls : ls + LB], p_pj[:])

            def proj_q_block(b, q_in):
                ls = b * LB
                p_pj = ps_pj.tile([128, LB], F32, tag="pj")
                for ec in range(NEC):
                    nc.tensor.matmul(
                        p_pj[:], wq_sb[:, ec, :], q_in[:, ec, :],
                        start=(ec == 0), stop=(ec == NEC - 1),
                    )
                nc.vector.tensor_copy(QT_sb[:, ls : ls + LB], p_pj[:])

            def proj_v_block(b, v_in):
                # project V^T (stationary weights), flip to [k, h] via PE
                ls = b * LB
                p_pj = ps_pj.tile([128, LB], F32, tag="pj")
                for ec in range(NEC):
                    nc.tensor.matmul(
                        p_pj[0:H, :], wv_sb[:, ec, :], v_in[:, ec, :],
                        start=(ec == 0), stop=(ec == NEC - 1),
                    )
                nc.vector.tensor_copy(VT_sb[:, ls : ls + LB], p_pj[0:H, :])
                for sub in range(4):
                    c = b * 4 + sub
                    p_tr = ps_tr.tile([128, H], F16, tag="pjt")
                    nc.tensor.transpose(
                        p_tr[:], VT_sb[:, c * 128 : (c + 1) * 128], id_sb[:]
                    )
                    nc.vector.tensor_copy(V_sb[:, c, 0:H], p_tr[:])

            def load_one(pool, tag, src):
                t = pool.tile([128, NEC, LB], F16, tag=tag)
                nc.sync.dma_start(t[:], src)
                return t

            def pool_convert(m8_sb, g):
                # fp8 -> f16 convert on Pool for tile g's chunk pair
                t = mcvpool.tile([128, 2 * LB], F16, tag="mcv")
                nc.gpsimd.tensor_copy(
                    t[:],
                    m8_sb[:, 2 * g : 2 * g + 2, :].rearrange("p c q -> p (c q)"),
                )
                return t

            # ---- minimal startup: K0,K1 + q0 + V0 ----
            k_in0 = load_one(kinp, "kin", k2[0])
            k_in1 = load_one(kinp, "kin", k2[1])
            q0 = load_one(qinp, "qin", q2[0])
            v_in0 = load_one(vinp, "vin", v2[0])
            v_in1 = load_one(vinp, "vin", v2[1])
            proj_k_block(0, k_in0)
            proj_k_block(1, k_in1)
            proj_q_block(0, q0)
            proj_v_block(0, v_in0)
            k_in2 = load_one(kinp, "kin", k2[2])

            # ---- main loop ----
            m8t, m16t = m8_sb0, m16_sb0
            q_next = None
            m8_next = m16_next = None
            k_pend = {2: k_in2}
            v_pend = {1: v_in1}
            mconv = {}  # pool-converted f16 mask tiles for current qb
            # pre-issue pool converts for first two pool-route tiles of qb0
            for g in range(N_PE, min(N_PE + 2, N_PE + N_POOL)):
                mconv[g] = pool_convert(m8t, g)

            for qb in range(NQB):
                qs = qb * LB
                p_oA = ps_oa.tile([128, LB], F32, tag="p_oA")
                p_oB = ps_ob.tile([128, LB], F32, tag="p_oB")
                for g in range(NG):
                    cA = 2 * g
                    route_pe = g < N_PE
                    route_pool = N_PE <= g < N_PE + N_POOL
                    route_d8 = N_PE + N_POOL <= g < N_PE + N_POOL + N_DVE8
                    ps = ps_st.tile([128, 1024], F32, tag="p_st")
                    if route_pe:
                        nc.tensor.matmul(
                            ps[:, 0:512],
                            id320_sb[:],
                            m8t[:, cA, :],
                            start=True, stop=False, skip_group_check=True,
                        )
                        nc.tensor.matmul(
                            ps[:, 512:1024],
                            id320_sb[:],
                            m8t[:, cA + 1, :],
                            start=True, stop=False, skip_group_check=True,
                        )
                    # two concurrent K=64 row-tiled score matmuls (N=512)
                    nc.tensor.matmul(
                        ps[:, 0:512],
                        KT_sb[0:64, cA * 128 : (cA + 1) * 128],
                        QT_sb[0:64, qs : qs + LB],
                        start=not route_pe, stop=True, skip_group_check=True,
                    )
                    nc.tensor.matmul(
                        ps[:, 512:1024],
                        KT_sb[64:128, (cA + 1) * 128 : (cA + 2) * 128],
                        QT_sb[64:128, qs : qs + LB],
                        start=not route_pe, stop=True, skip_group_check=True,
                    )
                    # streaming projections during qb0
                    if qb == 0:
                        if g % 2 == 0:
                            b = g // 2 + 2
                            if b <= 7:
                                proj_k_block(b, k_pend.pop(b))
                            if b + 1 <= 7:
                                k_pend[b + 1] = load_one(kinp, "kin", k2[b + 1])
                        else:
                            b = (g + 1) // 2
                            if b <= 7:
                                proj_v_block(b, v_pend.pop(b))
                            if b + 1 <= 7:
                                v_pend[b + 1] = load_one(vinp, "vin", v2[b + 1])
                    # exp on ACT (with -40 bias for PE-route tiles)
                    pt = ptpool.tile([128, 1024], F16, tag="pt")
                    if route_pe:
                        nc.scalar.activation(
                            pt[:], ps[:], EXP, scale=0.125, bias=bneg40[:]
                        )
                    else:
                        nc.scalar.activation(pt[:], ps[:], EXP, scale=0.125)
                    # mask multiply on DVE for non-PE routes
                    if route_pool:
                        nc.vector.tensor_mul(pt[:], pt[:], mconv.pop(g)[:])
                    elif route_d8:
                        nc.vector.tensor_mul(
                            pt[:],
                            pt[:],
                            m8t[:, cA : cA + 2, :].rearrange("p c q -> p (c q)"),
                        )
                    elif not route_pe:
                        cf = cA - NF8C
                        nc.vector.tensor_mul(
                            pt[:],
                            pt[:],
                            m16t[:, cf : cf + 2, :].rearrange("p c q -> p (c q)"),
                        )
                    # pool converts, two tiles ahead
                    gc = g + 2
                    if N_PE <= gc < N_PE + N_POOL:
                        mconv[gc] = pool_convert(m8t, gc)
                    # prefetch hooks for qb+1
                    if qb + 1 < NQB:
                        if g == 1:
                            m8_next = m8pool.tile([128, NF8C, LB], F8, tag="m8")
                            nc.sync.dma_start(m8_next[:], m8[qb + 1])
                        if g == 2:
                            m16_next = m16pool.tile([128, NF16C, LB], F16, tag="m16")
                            nc.sync.dma_start(m16_next[:], m16[qb + 1])
                        if g == 4:
                            q_next = load_one(qinp, "qin", q2[qb + 1])
                        if g == 8:
                            proj_q_block(qb + 1, q_next)
                    # AV: two concurrent K=64 row-split pairs per chunk
                    for ci, c in enumerate((cA, cA + 1)):
                        cols = slice(512 * ci, 512 * (ci + 1))
                        first = g == 0 and ci == 0
                        last = g == NG - 1 and ci == 1
                        nc.tensor.matmul(
                            p_oA[:], V_sb[0:64, c, :], pt[0:64, cols],
                            start=first, stop=last,
                        )
                        nc.tensor.matmul(
                            p_oB[:], V_sb[64:128, c, :], pt[64:128, cols],
                            start=first, stop=last,
                        )
                # epilogue: combine halves, ship unnormalized [O^T; Z] twice
                stg = opool.tile([H + 1, LB], F32, tag="ostg")
                nc.vector.tensor_copy(stg[:], p_oA[0 : H + 1, :])
                o_sb = opool.tile([H + 1, LB], F32, tag="osb")
                nc.vector.tensor_add(o_sb[:], stg[:], p_oB[0 : H + 1, :])
                nc.sync.dma_start(out[:, qs : qs + LB], o_sb[:])
                # next qb's pool converts for the first two pool tiles
                if qb + 1 < NQB:
                    for gg in range(N_PE, min(N_PE + 2, N_PE + N_POOL)):
                        mconv[gg] = pool_convert(m8_next, gg)
                m8t, m16t = m8_next, m16_next
    nc.compile()
    return nc


_NC_CACHE = {}


def _shuffle_pcl(xT):
    """xT: [E, L] -> [NQB, 128, NEC, LB]."""
    a = xT.reshape(NEC, 128, NQB, LB)
    return np.ascontiguousarray(a.transpose(2, 1, 0, 3))


def _shuffle_mask(forb_b):
    """forb_b: [L, L] bool (True = forbidden) -> [NQB, 128, NCH, LB] u8
    allowed mask: [qb, p, c, q'] = 1 - forb[qb*512+q', c*128+p]."""
    A = forb_b.T.reshape(NCH, 128, NQB, LB)
    return (1 - np.ascontiguousarray(A.transpose(2, 1, 0, 3))).astype(np.uint8)


def kernel(query, key, value, mask, WQ, WK, WV):
    if "nc" not in _NC_CACHE:
        _NC_CACHE["nc"] = build_nc()
    nc = _NC_CACHE["nc"]

    wqT = np.asarray(WQ, dtype=np.float16).T  # [E, H]
    wkT = np.asarray(WK, dtype=np.float16).T
    wvT = np.ascontiguousarray(np.asarray(WV, dtype=np.float16).T)
    wqD = np.ascontiguousarray(np.concatenate([wqT, wqT], axis=1))
    wkD = np.ascontiguousarray(np.concatenate([wkT, wkT], axis=1))
    idn = np.eye(64, dtype=np.float16)
    id320 = (MB * np.eye(128)).astype(np.float16)
    forb = np.asarray(mask)  # [B, L, L], True where forbidden
    in_maps = []
    for b in range(B):
        allow = _shuffle_mask(forb[b])  # [NQB, 128, NCH, LB] u8 {0,1}
        a8 = allow[:, :, :NF8C, :]
        a16 = allow[:, :, NF8C:, :]
        m8b = np.where(a8 > 0, np.uint8(0x38), np.uint8(0)).view(
            ml_dtypes.float8_e4m3
        )
        in_maps.append(
            {
                "q2": _shuffle_pcl(np.asarray(query[b], dtype=np.float16).T),
                "k2": _shuffle_pcl(np.asarray(key[b], dtype=np.float16).T),
                "v2": _shuffle_pcl(np.asarray(value[b], dtype=np.float16).T),
                "m8": np.ascontiguousarray(m8b),
                "m16": np.ascontiguousarray(a16.astype(np.float16)),
                "wqD": wqD,
                "wkD": wkD,
                "wvT": wvT,
                "ident": idn,
                "id320": id320,
            }
        )
    res = bass_utils.run_bass_kernel_spmd(nc, in_maps, core_ids=list(range(NCORES)))
    outs = []
    for b in range(B):
        ot = res.results[b]["out"].astype(np.float64)  # [65, L] combined
        o = (ot[0:H] / ot[H : H + 1]).T  # [L, H]
        outs.append(o.astype(np.float32))
    return np.stack(outs, axis=0)


if __name__ == "__main__":
    rng = np.random.default_rng(0)
    q = rng.standard_normal((B, L, E), dtype=np.float32)
    k = rng.standard_normal((B, L, E), dtype=np.float32)
    v = rng.standard_normal((B, L, E), dtype=np.float32)
    m = rng.integers(0, 2, size=(B, L, L)).astype(bool)
    s = 1.0 / np.sqrt(E)
    wq = rng.uniform(-s, s, size=(H, E)).astype(np.float32)
    wk = rng.uniform(-s, s, size=(H, E)).astype(np.float32)
    wv = rng.uniform(-s, s, size=(H, E)).astype(np.float32)
    o = kernel(query=q, key=k, value=v, mask=m, WQ=wq, WK=wk, WV=wv)
    print(o.shape, o.dtype)


# revision 10
# speedup vs baseline: 1.3264x; 1.3264x over previous
"""Fused masked-attention kernel for Trainium2, data-parallel over batch on 8 cores.

v17 design notes (all per core; one batch element per core):
- Steady state identical to the proven v15 shape: per tile (chunk pair)
  score-pair matmuls (row-split K=64 concurrent), ACT exp (the wall,
  ~1.13us/tile), DVE mask multiply, serial AV accumulation into one PSUM
  bank.
- Mask DMA cut by shipping HALF the chunks as fp8e4 {0,1} (1B/elem):
    * 4 tiles/qb converted fp8->f16 on the Pool engine (measured 3.6us/tile,
      issued ~4 beats ahead), then normal f16 2x DVE multiply;
    * 4 tiles/qb multiplied directly from fp8 on DVE (1x mode, ~1.2us);
    * 8 tiles/qb stay f16 over DMA (2x DVE multiply).
  Total DMA drops 60.5MB -> ~48MB, below the ACT roofline.
- Startup streams: only K0,K1 + V0,V1 + q0 + mask0 load before the main
  loop; K blocks 2..7 and V blocks 1..7 DMA+project inside qb0's slots
  just ahead of first use, so the exp stream starts ~15us in instead of
  ~50us.
- Output ships combined unnormalized O^T+Z rows; host does divide+transpose.
"""

import numpy as np
import ml_dtypes

import concourse.bass as bass
import concourse.tile as tile
from concourse import bacc, mybir
from concourse import bass_utils

B, L, E, H = 8, 4096, 1024, 64
NCORES = 8
F32 = mybir.dt.float32
F16 = mybir.dt.float16
F8 = mybir.dt.float8e4

LB = 512           # q-block and projection block width
NQB = L // LB      # 8
NCH = L // 128     # 32 k-chunks
NEC = E // 128     # 8 e-chunks
NG = 16            # tiles (chunk pairs) per q-block

# mask route knobs (per qb, sum must be NG); fp8 tiles first, f16 tiles last
N_POOL = 4         # fp8 -> f16 on Pool, f16 mult on DVE
N_DVE8 = 4         # direct fp8 mult on DVE (1x)
N_F16 = 8          # f16 mask over DMA, f16 mult on DVE (2x)
assert N_POOL + N_DVE8 + N_F16 == NG
NF8C = 2 * (N_POOL + N_DVE8)   # fp8 chunks per qb
NF16C = 2 * N_F16              # f16 chunks per qb


def build_nc():
    nc = bacc.Bacc(
        "TRN2",
        target_bir_lowering=False,
        debug=False,
        enable_asserts=False,
        num_devices=NCORES,
    )
    q2 = nc.dram_tensor("q2", [NQB, 128, NEC, LB], F16, kind="ExternalInput").ap()
    k2 = nc.dram_tensor("k2", [NQB, 128, NEC, LB], F16, kind="ExternalInput").ap()
    v2 = nc.dram_tensor("v2", [NQB, 128, NEC, LB], F16, kind="ExternalInput").ap()
    m8 = nc.dram_tensor("m8", [NQB, 128, NF8C, LB], F8, kind="ExternalInput").ap()
    m16 = nc.dram_tensor("m16", [NQB, 128, NF16C, LB], F16, kind="ExternalInput").ap()
    wqD = nc.dram_tensor("wqD", [E, 128], F16, kind="ExternalInput").ap()
    wkD = nc.dram_tensor("wkD", [E, 128], F16, kind="ExternalInput").ap()
    wvT = nc.dram_tensor("wvT", [E, H], F16, kind="ExternalInput").ap()
    ident = nc.dram_tensor("ident", [64, 64], F16, kind="ExternalInput").ap()
    out = nc.dram_tensor("out", [H + 1, L], F32, kind="ExternalOutput").ap()

    EXP = mybir.ActivationFunctionType.Exp

    with tile.TileContext(nc) as tc:
        with (
            tc.tile_pool(name="const", bufs=1) as constp,
            tc.tile_pool(name="persist", bufs=1) as persist,
            tc.tile_pool(name="kin", bufs=2) as kinp,
            tc.tile_pool(name="vin", bufs=2) as vinp,
            tc.tile_pool(name="qin", bufs=2) as qinp,
            tc.tile_pool(name="m8p", bufs=2) as m8pool,
            tc.tile_pool(name="m16p", bufs=2) as m16pool,
            tc.tile_pool(name="mcv", bufs=5) as mcvpool,
            tc.tile_pool(name="pt", bufs=10) as ptpool,
            tc.tile_pool(name="osb", bufs=2) as opool,
            tc.tile_pool(name="ps_st", bufs=2, space="PSUM") as ps_st,
            tc.tile_pool(name="ps_o", bufs=1, space="PSUM") as ps_o,
            tc.tile_pool(name="ps_pj", bufs=2, space="PSUM") as ps_pj,
        ):
            # ---- constants / weights ----
            wq_sb = constp.tile([128, NEC, 128], F16)
            wk_sb = constp.tile([128, NEC, 128], F16)
            wv_sb = constp.tile([128, NEC, H], F16)
            nc.sync.dma_start(wq_sb[:], wqD.rearrange("(c p) h -> p c h", p=128))
            nc.sync.dma_start(wk_sb[:], wkD.rearrange("(c p) h -> p c h", p=128))
            nc.sync.dma_start(wv_sb[:], wvT.rearrange("(c p) h -> p c h", p=128))
            id_sb = constp.tile([64, 64], F16)
            nc.sync.dma_start(id_sb[:], ident)

            # persistent projected tensors
            QT_sb = persist.tile([128, L], F16)   # rows 0:64 = Q^T, 64:128 copy
            KT_sb = persist.tile([128, L], F16)
            VT_sb = persist.tile([64, L], F16)    # V^T staging
            V_sb = persist.tile([128, NCH, 128], F16)  # [k, h] + ones col 64
            nc.vector.memset(V_sb[:, :, H : 128], 0.0)
            nc.vector.memset(V_sb[:, :, H : H + 1], 1.0)

            # masks for qb0
            m8_sb0 = m8pool.tile([128, NF8C, LB], F8, tag="m8")
            nc.sync.dma_start(m8_sb0[:], m8[0])
            m16_sb0 = m16pool.tile([128, NF16C, LB], F16, tag="m16")
            nc.sync.dma_start(m16_sb0[:], m16[0])

            # ---- PE warmup on weights (HAM) ----
            p_w = ps_st.tile([128, 1024], F32, tag="p_st")
            for w in range(110):
                nc.tensor.matmul(
                    p_w[:, 0:128], wq_sb[:, 0, :], wq_sb[:, 0, 0:128],
                    start=True, stop=True,
                )

            def proj_k_block(b, k_in):
                ls = b * LB
                p_pj = ps_pj.tile([128, LB], F32, tag="pj")
                for ec in range(NEC):
                    nc.tensor.matmul(
                        p_pj[:], wk_sb[:, ec, :], k_in[:, ec, :],
                        start=(ec == 0), stop=(ec == NEC - 1),
                    )
                nc.vector.tensor_copy(KT_sb[:, ls : ls + LB], p_pj[:])

            def proj_q_block(b, q_in):
                ls = b * LB
                p_pj = ps_pj.tile([128, LB], F32, tag="pj")
                for ec in range(NEC):
                    nc.tensor.matmul(
                        p_pj[:], wq_sb[:, ec, :], q_in[:, ec, :],
                        start=(ec == 0), stop=(ec == NEC - 1),
                    )
                nc.vector.tensor_copy(QT_sb[:, ls : ls + LB], p_pj[:])

            def proj_v_block(b, v_in):
                ls = b * LB
                p_pj = ps_pj.tile([128, LB], F32, tag="pj")
                for ec in range(NEC):
                    nc.tensor.matmul(
                        p_pj[0:H, :], wv_sb[:, ec, :], v_in[:, ec, :],
                        start=(ec == 0), stop=(ec == NEC - 1),
                    )
                nc.vector.tensor_copy(VT_sb[:, ls : ls + LB], p_pj[0:H, :])
                for sub in range(4):
                    c = b * 4 + sub
                    p_tr = ps_o.tile([128, H], F16, tag="pjt")
                    nc.tensor.transpose(
                        p_tr[:], VT_sb[:, c * 128 : (c + 1) * 128], id_sb[:]
                    )
                    nc.vector.tensor_copy(V_sb[:, c, 0:H], p_tr[:])

            def load_one(pool, tag, src):
                t = pool.tile([128, NEC, LB], F16, tag=tag)
                nc.sync.dma_start(t[:], src)
                return t

            def pool_convert(m8_sb, g):
                # fp8 -> f16 convert on Pool for tile g's chunk pair
                t = mcvpool.tile([128, 2 * LB], F16, tag="mcv")
                nc.gpsimd.tensor_copy(
                    t[:],
                    m8_sb[:, 2 * g : 2 * g + 2, :].rearrange("p c q -> p (c q)"),
                )
                return t

            # ---- minimal startup: K0,K1 + q0 + V0 ----
            k_in0 = load_one(kinp, "kin", k2[0])
            k_in1 = load_one(kinp, "kin", k2[1])
            q0 = load_one(qinp, "qin", q2[0])
            v_in0 = load_one(vinp, "vin", v2[0])
            v_in1 = load_one(vinp, "vin", v2[1])
            proj_k_block(0, k_in0)
            proj_k_block(1, k_in1)
            proj_q_block(0, q0)
            proj_v_block(0, v_in0)
            k_in2 = load_one(kinp, "kin", k2[2])

            # ---- main loop ----
            m8t, m16t = m8_sb0, m16_sb0
            q_next = None
            m8_next = m16_next = None
            k_pend = {2: k_in2}
            v_pend = {1: v_in1}
            mconv = {}
            for g in range(min(4, N_POOL)):
                mconv[g] = pool_convert(m8t, g)

            for qb in range(NQB):
                qs = qb * LB
                p_o = ps_o.tile([128, LB], F32, tag="p_o")
                for g in range(NG):
                    cA, cB = 2 * g, 2 * g + 1
                    route_pool = g < N_POOL
                    route_d8 = N_POOL <= g < N_POOL + N_DVE8
                    ps = ps_st.tile([128, 1024], F32, tag="p_st")
                    # two concurrent K=64 row-tiled score matmuls (N=512)
                    nc.tensor.matmul(
                        ps[:, 0:512],
                        KT_sb[0:64, cA * 128 : (cA + 1) * 128],
                        QT_sb[0:64, qs : qs + LB],
                        start=True, stop=True,
                    )
                    nc.tensor.matmul(
                        ps[:, 512:1024],
                        KT_sb[64:128, cB * 128 : (cB + 1) * 128],
                        QT_sb[64:128, qs : qs + LB],
                        start=True, stop=True,
                    )
                    # streaming projections during qb0
                    if qb == 0:
                        if g % 2 == 0:
                            b = g // 2 + 2
                            if b <= 7:
                                proj_k_block(b, k_pend.pop(b))
                            if b + 1 <= 7:
                                k_pend[b + 1] = load_one(kinp, "kin", k2[b + 1])
                        else:
                            b = (g + 1) // 2
                            if b <= 7:
                                proj_v_block(b, v_pend.pop(b))
                            if b + 1 <= 7:
                                v_pend[b + 1] = load_one(vinp, "vin", v2[b + 1])
                    # exp on ACT
                    pt = ptpool.tile([128, 1024], F16, tag="pt")
                    nc.scalar.activation(pt[:], ps[:], EXP, scale=0.125)
                    # mask multiply on DVE
                    if route_pool:
                        nc.vector.tensor_mul(pt[:], pt[:], mconv.pop(g)[:])
                    elif route_d8:
                        nc.vector.tensor_mul(
                            pt[:],
                            pt[:],
                            m8t[:, cA : cA + 2, :].rearrange("p c q -> p (c q)"),
                        )
                    else:
                        cf = cA - NF8C
                        nc.vector.tensor_mul(
                            pt[:],
                            pt[:],
                            m16t[:, cf : cf + 2, :].rearrange("p c q -> p (c q)"),
                        )
                    # prefetch hooks for qb+1
                    if qb + 1 < NQB:
                        if g == 1:
                            m8_next = m8pool.tile([128, NF8C, LB], F8, tag="m8")
                            nc.sync.dma_start(m8_next[:], m8[qb + 1])
                        if g == 2:
                            m16_next = m16pool.tile([128, NF16C, LB], F16, tag="m16")
                            nc.sync.dma_start(m16_next[:], m16[qb + 1])
                        if g == 4:
                            q_next = load_one(qinp, "qin", q2[qb + 1])
                        if g == 8:
                            proj_q_block(qb + 1, q_next)
                        # next qb's pool converts, early enough for Pool's
                        # ~3.6us/tile serial rate
                        if 6 <= g < 6 + N_POOL:
                            mconv[g - 6] = pool_convert(m8_next, g - 6)
                    # AV: accumulate both chunks
                    nc.tensor.matmul(
                        p_o[:], V_sb[:, cA, :], pt[:, 0:512],
                        start=(g == 0), stop=False,
                    )
                    nc.tensor.matmul(
                        p_o[:], V_sb[:, cB, :], pt[:, 512:1024],
                        start=False, stop=(g == NG - 1),
                    )
                # epilogue: ship unnormalized O^T + Z row
                o_sb = opool.tile([H + 1, LB], F32, tag="osb")
                nc.vector.tensor_copy(o_sb[:], p_o[0 : H + 1, :])
                nc.sync.dma_start(out[:, qs : qs + LB], o_sb[:])
                m8t, m16t = m8_next, m16_next
    nc.compile()
    return nc


_NC_CACHE = {}


def _shuffle_pcl(xT):
    """xT: [E, L] -> [NQB, 128, NEC, LB]."""
    a = xT.reshape(NEC, 128, NQB, LB)
    return np.ascontiguousarray(a.transpose(2, 1, 0, 3))


def _shuffle_mask(forb_b):
    """forb_b: [L, L] bool (True = forbidden) -> [NQB, 128, NCH, LB] u8
    allowed mask: [qb, p, c, q'] = 1 - forb[qb*512+q', c*128+p]."""
    A = forb_b.T.reshape(NCH, 128, NQB, LB)
    return (1 - np.ascontiguousarray(A.transpose(2, 1, 0, 3))).astype(np.uint8)


def kernel(query, key, value, mask, WQ, WK, WV):
    if "nc" not in _NC_CACHE:
        _NC_CACHE["nc"] = build_nc()
    nc = _NC_CACHE["nc"]

    wqT = np.asarray(WQ, dtype=np.float16).T  # [E, H]
    wkT = np.asarray(WK, dtype=np.float16).T
    wvT = np.ascontiguousarray(np.asarray(WV, dtype=np.float16).T)
    wqD = np.ascontiguousarray(np.concatenate([wqT, wqT], axis=1))
    wkD = np.ascontiguousarray(np.concatenate([wkT, wkT], axis=1))
    idn = np.eye(64, dtype=np.float16)
    forb = np.asarray(mask)  # [B, L, L], True where forbidden
    in_maps = []
    for b in range(B):
        allow = _shuffle_mask(forb[b])  # [NQB, 128, NCH, LB] u8 {0,1}
        a8 = allow[:, :, :NF8C, :]
        a16 = allow[:, :, NF8C:, :]
        m8b = np.where(a8 > 0, np.uint8(0x38), np.uint8(0)).view(
            ml_dtypes.float8_e4m3
        )
        in_maps.append(
            {
                "q2": _shuffle_pcl(np.asarray(query[b], dtype=np.float16).T),
                "k2": _shuffle_pcl(np.asarray(key[b], dtype=np.float16).T),
                "v2": _shuffle_pcl(np.asarray(value[b], dtype=np.float16).T),
                "m8": np.ascontiguousarray(m8b),
                "m16": np.ascontiguousarray(a16.astype(np.float16)),
                "wqD": wqD,
                "wkD": wkD,
                "wvT": wvT,
                "ident": idn,
            }
        )
    res = bass_utils.run_bass_kernel_spmd(nc, in_maps, core_ids=list(range(NCORES)))
    outs = []
    for b in range(B):
        ot = res.results[b]["out"].astype(np.float64)  # [65, L]
        o = (ot[0:H] / ot[H : H + 1]).T  # [L, H]
        outs.append(o.astype(np.float32))
    return np.stack(outs, axis=0)


if __name__ == "__main__":
    rng = np.random.default_rng(0)
    q = rng.standard_normal((B, L, E), dtype=np.float32)
    k = rng.standard_normal((B, L, E), dtype=np.float32)
    v = rng.standard_normal((B, L, E), dtype=np.float32)
    m = rng.integers(0, 2, size=(B, L, L)).astype(bool)
    s = 1.0 / np.sqrt(E)
    wq = rng.uniform(-s, s, size=(H, E)).astype(np.float32)
    wk = rng.uniform(-s, s, size=(H, E)).astype(np.float32)
    wv = rng.uniform(-s, s, size=(H, E)).astype(np.float32)
    o = kernel(query=q, key=k, value=v, mask=m, WQ=wq, WK=wk, WV=wv)
    print(o.shape, o.dtype)


# revision 11
# speedup vs baseline: 1.8228x; 1.3743x over previous
"""Fused masked-attention kernel for Trainium2, data-parallel over batch on 8 cores.

v18 design notes (all per core; one batch element per core):
- Steady state identical to the proven v15 shape: per tile (chunk pair)
  score-pair matmuls (row-split K=64 concurrent), ACT exp (the wall,
  ~1.13us/tile), DVE f16 2x-mode mask multiply, serial AV accumulation
  into one PSUM bank. All-f16 mask: measured power throttling (activity_1
  caps engine util at 50%) punishes extra engine work (Pool casts, 1x-mode
  DVE fp8 multiplies) more than the DMA bytes cost.
- Startup streams instead of serializing: only K0,K1 + V0,V1 + q0 load
  before the main loop (issued ahead of the mask DMA); K blocks 2..7 and
  V blocks 1..7 DMA+project inside qb0's slots just ahead of first use,
  so the exp stream starts ~15us in instead of ~50us.
- Output ships unnormalized O^T+Z rows; host does divide+transpose.
"""

import numpy as np

import concourse.bass as bass
import concourse.tile as tile
from concourse import bacc, mybir
from concourse import bass_utils

B, L, E, H = 8, 4096, 1024, 64
NCORES = 8
F32 = mybir.dt.float32
F16 = mybir.dt.float16

LB = 512           # q-block and projection block width
NQB = L // LB      # 8
NCH = L // 128     # 32 k-chunks
NEC = E // 128     # 8 e-chunks
NG = 16            # tiles (chunk pairs) per q-block


def build_nc():
    nc = bacc.Bacc(
        "TRN2",
        target_bir_lowering=False,
        debug=False,
        enable_asserts=False,
        num_devices=NCORES,
    )
    q2 = nc.dram_tensor("q2", [NQB, 128, NEC, LB], F16, kind="ExternalInput").ap()
    k2 = nc.dram_tensor("k2", [NQB, 128, NEC, LB], F16, kind="ExternalInput").ap()
    v2 = nc.dram_tensor("v2", [NQB, 128, NEC, LB], F16, kind="ExternalInput").ap()
    mu8 = nc.dram_tensor("mu8", [NQB, 128, NCH, LB], F16, kind="ExternalInput").ap()
    wqD = nc.dram_tensor("wqD", [E, 128], F16, kind="ExternalInput").ap()
    wkD = nc.dram_tensor("wkD", [E, 128], F16, kind="ExternalInput").ap()
    wvT = nc.dram_tensor("wvT", [E, H], F16, kind="ExternalInput").ap()
    ident = nc.dram_tensor("ident", [64, 64], F16, kind="ExternalInput").ap()
    out = nc.dram_tensor("out", [H + 1, L], F32, kind="ExternalOutput").ap()

    EXP = mybir.ActivationFunctionType.Exp

    with tile.TileContext(nc) as tc:
        with (
            tc.tile_pool(name="const", bufs=1) as constp,
            tc.tile_pool(name="persist", bufs=1) as persist,
            tc.tile_pool(name="kin", bufs=2) as kinp,
            tc.tile_pool(name="vin", bufs=2) as vinp,
            tc.tile_pool(name="qin", bufs=2) as qinp,
            tc.tile_pool(name="mpk", bufs=2) as mpool,
            tc.tile_pool(name="pt", bufs=10) as ptpool,
            tc.tile_pool(name="osb", bufs=2) as opool,
            tc.tile_pool(name="ps_st", bufs=2, space="PSUM") as ps_st,
            tc.tile_pool(name="ps_o", bufs=1, space="PSUM") as ps_o,
            tc.tile_pool(name="ps_pj", bufs=2, space="PSUM") as ps_pj,
        ):
            # ---- constants / weights ----
            wq_sb = constp.tile([128, NEC, 128], F16)
            wk_sb = constp.tile([128, NEC, 128], F16)
            wv_sb = constp.tile([128, NEC, H], F16)
            nc.sync.dma_start(wq_sb[:], wqD.rearrange("(c p) h -> p c h", p=128))
            nc.sync.dma_start(wk_sb[:], wkD.rearrange("(c p) h -> p c h", p=128))
            nc.sync.dma_start(wv_sb[:], wvT.rearrange("(c p) h -> p c h", p=128))
            id_sb = constp.tile([64, 64], F16)
            nc.sync.dma_start(id_sb[:], ident)

            # persistent projected tensors
            QT_sb = persist.tile([128, L], F16)   # rows 0:64 = Q^T, 64:128 copy
            KT_sb = persist.tile([128, L], F16)
            VT_sb = persist.tile([64, L], F16)    # V^T staging
            V_sb = persist.tile([128, NCH, 128], F16)  # [k, h] + ones col 64
            nc.vector.memset(V_sb[:, :, H : 128], 0.0)
            nc.vector.memset(V_sb[:, :, H : H + 1], 1.0)

            def proj_k_block(b, k_in):
                ls = b * LB
                p_pj = ps_pj.tile([128, LB], F32, tag="pj")
                for ec in range(NEC):
                    nc.tensor.matmul(
                        p_pj[:], wk_sb[:, ec, :], k_in[:, ec, :],
                        start=(ec == 0), stop=(ec == NEC - 1),
                    )
                nc.vector.tensor_copy(KT_sb[:, ls : ls + LB], p_pj[:])

            def proj_q_block(b, q_in):
                ls = b * LB
                p_pj = ps_pj.tile([128, LB], F32, tag="pj")
                for ec in range(NEC):
                    nc.tensor.matmul(
                        p_pj[:], wq_sb[:, ec, :], q_in[:, ec, :],
                        start=(ec == 0), stop=(ec == NEC - 1),
                    )
                nc.vector.tensor_copy(QT_sb[:, ls : ls + LB], p_pj[:])

            def proj_v_block(b, v_in):
                ls = b * LB
                p_pj = ps_pj.tile([128, LB], F32, tag="pj")
                for ec in range(NEC):
                    nc.tensor.matmul(
                        p_pj[0:H, :], wv_sb[:, ec, :], v_in[:, ec, :],
                        start=(ec == 0), stop=(ec == NEC - 1),
                    )
                nc.vector.tensor_copy(VT_sb[:, ls : ls + LB], p_pj[0:H, :])
                for sub in range(4):
                    c = b * 4 + sub
                    p_tr = ps_o.tile([128, H], F16, tag="pjt")
                    nc.tensor.transpose(
                        p_tr[:], VT_sb[:, c * 128 : (c + 1) * 128], id_sb[:]
                    )
                    nc.vector.tensor_copy(V_sb[:, c, 0:H], p_tr[:])

            def load_one(pool, tag, src):
                t = pool.tile([128, NEC, LB], F16, tag=tag)
                nc.sync.dma_start(t[:], src)
                return t

            # ---- minimal startup: K0,K1 + q0 + V0,V1 ahead of masks ----
            k_in0 = load_one(kinp, "kin", k2[0])
            k_in1 = load_one(kinp, "kin", k2[1])
            q0 = load_one(qinp, "qin", q2[0])
            v_in0 = load_one(vinp, "vin", v2[0])
            mpk_sb0 = mpool.tile([128, NCH, LB], F16, tag="mpk")
            nc.sync.dma_start(mpk_sb0[:], mu8[0])
            v_in1 = load_one(vinp, "vin", v2[1])

            # ---- PE warmup on weights (HAM) ----
            p_w = ps_st.tile([128, 1024], F32, tag="p_st")
            for w in range(110):
                nc.tensor.matmul(
                    p_w[:, 0:128], wq_sb[:, 0, :], wq_sb[:, 0, 0:128],
                    start=True, stop=True,
                )

            proj_k_block(0, k_in0)
            proj_k_block(1, k_in1)
            proj_q_block(0, q0)
            proj_v_block(0, v_in0)
            k_in2 = load_one(kinp, "kin", k2[2])

            # ---- main loop ----
            mtile = mpk_sb0
            q_next = None
            m_next = None
            k_pend = {2: k_in2}
            v_pend = {1: v_in1}
            for qb in range(NQB):
                qs = qb * LB
                p_o = ps_o.tile([128, LB], F32, tag="p_o")
                for g in range(NG):
                    cA, cB = 2 * g, 2 * g + 1
                    ps = ps_st.tile([128, 1024], F32, tag="p_st")
                    # two concurrent K=64 row-tiled score matmuls (N=512)
                    nc.tensor.matmul(
                        ps[:, 0:512],
                        KT_sb[0:64, cA * 128 : (cA + 1) * 128],
                        QT_sb[0:64, qs : qs + LB],
                        start=True, stop=True,
                    )
                    nc.tensor.matmul(
                        ps[:, 512:1024],
                        KT_sb[64:128, cB * 128 : (cB + 1) * 128],
                        QT_sb[64:128, qs : qs + LB],
                        start=True, stop=True,
                    )
                    # streaming projections during qb0
                    if qb == 0:
                        if g % 2 == 0:
                            b = g // 2 + 2
                            if b <= 7:
                                proj_k_block(b, k_pend.pop(b))
                            if b + 1 <= 7:
                                k_pend[b + 1] = load_one(kinp, "kin", k2[b + 1])
                        else:
                            b = (g + 1) // 2
                            if b <= 7:
                                proj_v_block(b, v_pend.pop(b))
                            if b + 1 <= 7:
                                v_pend[b + 1] = load_one(vinp, "vin", v2[b + 1])
                    # exp on ACT
                    pt = ptpool.tile([128, 1024], F16, tag="pt")
                    nc.scalar.activation(pt[:], ps[:], EXP, scale=0.125)
                    # mask-mult, all-f16 SBUF operands (DVE 2x mode)
                    nc.vector.tensor_mul(
                        pt[:],
                        pt[:],
                        mtile[:, cA : cA + 2, :].rearrange("p c q -> p (c q)"),
                    )
                    # prefetch hooks
                    if qb + 1 < NQB:
                        if g == 1:
                            m_next = mpool.tile([128, NCH, LB], F16, tag="mpk")
                            nc.sync.dma_start(m_next[:], mu8[qb + 1])
                        if g == 4:
                            q_next = load_one(qinp, "qin", q2[qb + 1])
                        if g == 8:
                            proj_q_block(qb + 1, q_next)
                    # AV: accumulate both chunks
                    nc.tensor.matmul(
                        p_o[:], V_sb[:, cA, :], pt[:, 0:512],
                        start=(g == 0), stop=False,
                    )
                    nc.tensor.matmul(
                        p_o[:], V_sb[:, cB, :], pt[:, 512:1024],
                        start=False, stop=(g == NG - 1),
                    )
                # epilogue: ship unnormalized O^T + Z row
                o_sb = opool.tile([H + 1, LB], F32, tag="osb")
                nc.vector.tensor_copy(o_sb[:], p_o[0 : H + 1, :])
                nc.sync.dma_start(out[:, qs : qs + LB], o_sb[:])
                mtile = m_next
    nc.compile()
    return nc


_NC_CACHE = {}


def _shuffle_pcl(xT):
    """xT: [E, L] -> [NQB, 128, NEC, LB]."""
    a = xT.reshape(NEC, 128, NQB, LB)
    return np.ascontiguousarray(a.transpose(2, 1, 0, 3))


def _shuffle_mask(forb_b):
    """forb_b: [L, L] bool (True = forbidden) -> [NQB, 128, NCH, LB] u8."""
    A = forb_b.T.reshape(NCH, 128, NQB, LB)
    return np.ascontiguousarray(A.transpose(2, 1, 0, 3)).astype(np.uint8)


def kernel(query, key, value, mask, WQ, WK, WV):
    if "nc" not in _NC_CACHE:
        _NC_CACHE["nc"] = build_nc()
    nc = _NC_CACHE["nc"]

    wqT = np.asarray(WQ, dtype=np.float16).T  # [E, H]
    wkT = np.asarray(WK, dtype=np.float16).T
    wvT = np.ascontiguousarray(np.asarray(WV, dtype=np.float16).T)
    wqD = np.ascontiguousarray(np.concatenate([wqT, wqT], axis=1))
    wkD = np.ascontiguousarray(np.concatenate([wkT, wkT], axis=1))
    idn = np.eye(64, dtype=np.float16)
    forb = np.asarray(mask)  # [B, L, L], True where forbidden
    in_maps = []
    for b in range(B):
        in_maps.append(
            {
                "q2": _shuffle_pcl(np.asarray(query[b], dtype=np.float16).T),
                "k2": _shuffle_pcl(np.asarray(key[b], dtype=np.float16).T),
                "v2": _shuffle_pcl(np.asarray(value[b], dtype=np.float16).T),
                "mu8": (1 - _shuffle_mask(forb[b])).astype(np.float16),
                "wqD": wqD,
                "wkD": wkD,
                "wvT": wvT,
                "ident": idn,
            }
        )
    res = bass_utils.run_bass_kernel_spmd(nc, in_maps, core_ids=list(range(NCORES)))
    outs = []
    for b in range(B):
        ot = res.results[b]["out"].astype(np.float64)  # [65, L]
        o = (ot[0:H] / ot[H : H + 1]).T  # [L, H]
        outs.append(o.astype(np.float32))
    return np.stack(outs, axis=0)


if __name__ == "__main__":
    rng = np.random.default_rng(0)
    q = rng.standard_normal((B, L, E), dtype=np.float32)
    k = rng.standard_normal((B, L, E), dtype=np.float32)
    v = rng.standard_normal((B, L, E), dtype=np.float32)
    m = rng.integers(0, 2, size=(B, L, L)).astype(bool)
    s = 1.0 / np.sqrt(E)
    wq = rng.uniform(-s, s, size=(H, E)).astype(np.float32)
    wk = rng.uniform(-s, s, size=(H, E)).astype(np.float32)
    wv = rng.uniform(-s, s, size=(H, E)).astype(np.float32)
    o = kernel(query=q, key=k, value=v, mask=m, WQ=wq, WK=wk, WV=wv)
    print(o.shape, o.dtype)
